# revision 5
# baseline (speedup 1.0000x reference)
"""Trainium2 Bass kernel: multi-head attention with Toeplitz relative bias.

Problem: B=16, L=1024, F=512, H=8, D=64 ViT patch attention.
Sharding: data-parallel over batch, 2 batches per core across 8 cores.

Device-side design (per core, fully unrolled Tile program):
  - Host pre-transposes inputs to xT [F, L] so the F-contraction of every
    projection has F on SBUF partitions with contiguous DMA loads.
  - qT/kT computed transposed ([fout, L], W chunks stationary); v computed
    natural ([L, fout], xT chunks stationary). All projections fp32 (f32r).
  - Scores computed transposed [k, q] (k on partitions) so the attn@v
    contraction needs no on-chip transpose. Softmax denominators come from
    a ones-column appended to V (U_aug row 64 = sum_k exp).
  - Toeplitz bias is pre-gathered on host to biasT[h,k,q] bf16; DVE adds it
    to scores (PSUM) while ACT does exp. Normalization multiplies by a
    PE-broadcast reciprocal row.
  - No max-subtraction in softmax: |scores| <~ 1.5 by construction
    (0.02-scale weights), exp is far from overflow.
"""

import os
import sys

import numpy as np

for _p in ("/opt/trn_rl_repo",):
    if _p not in sys.path:
        sys.path.insert(0, _p)

import ml_dtypes

import concourse.bass as bass
import concourse.mybir as mybir
import concourse.tile as tile
from concourse import bacc
from concourse.bass_utils import run_bass_kernel_spmd

B, L, F, H, D = 16, 1024, 512, 8, 64
NX, NY = 32, 32
NCORES = 8
BPC = B // NCORES  # batches per core
FP32 = mybir.dt.float32
F32R = mybir.dt.float32r
BF16 = mybir.dt.bfloat16
Exp = mybir.ActivationFunctionType.Exp
Identity = mybir.ActivationFunctionType.Identity
Add = mybir.AluOpType.add
Mult = mybir.AluOpType.mult


def _build():
    nc = bacc.Bacc("TRN2", target_bir_lowering=False, debug=False)

    xqT_d = nc.dram_tensor("xqT", [BPC, F, L], F32R, kind="ExternalInput").ap()
    xkvT_d = nc.dram_tensor("xkvT", [BPC, F, L], F32R, kind="ExternalInput").ap()
    Wq_d = nc.dram_tensor("Wq", [F, F], F32R, kind="ExternalInput").ap()
    Wk_d = nc.dram_tensor("Wk", [F, F], F32R, kind="ExternalInput").ap()
    Wv_d = nc.dram_tensor("Wv", [F, F], F32R, kind="ExternalInput").ap()
    Wo_d = nc.dram_tensor("Wo", [F, F], BF16, kind="ExternalInput").ap()
    bq_d = nc.dram_tensor("bq", [F], FP32, kind="ExternalInput").ap()
    bk_d = nc.dram_tensor("bk", [F], FP32, kind="ExternalInput").ap()
    bv_d = nc.dram_tensor("bv", [F], FP32, kind="ExternalInput").ap()
    bo_d = nc.dram_tensor("bo", [F], F32R, kind="ExternalInput").ap()
    biasT_d = nc.dram_tensor("biasT", [H, L, L], BF16, kind="ExternalInput").ap()
    ones_d = nc.dram_tensor("ones", [128], F32R, kind="ExternalInput").ap()
    out_d = nc.dram_tensor("out", [BPC, L, F], FP32, kind="ExternalOutput").ap()

    with tile.TileContext(nc) as tc:
        with (
            tc.tile_pool(name="const", bufs=1) as cpool,
            tc.tile_pool(name="xin", bufs=1) as xpool,
            tc.tile_pool(name="qkv", bufs=2) as qpool,
            tc.tile_pool(name="bias", bufs=2) as bpool,
            tc.tile_pool(name="work", bufs=4) as wpool,
            tc.tile_pool(name="exp", bufs=10) as epool,
            tc.tile_pool(name="psA", bufs=4, space="PSUM") as psA,
            tc.tile_pool(name="psU", bufs=2, space="PSUM") as psU,
            tc.tile_pool(name="psR", bufs=2, space="PSUM") as psR,
        ):
            # ---- constants: weights, biases, ones ----
            Wq_s = cpool.tile([128, 4 * F], F32R, tag="Wq")
            Wk_s = cpool.tile([128, 4 * F], F32R, tag="Wk")
            Wv_s = cpool.tile([128, 4 * F], F32R, tag="Wv")
            Wo_s = cpool.tile([128, 4 * F], BF16, tag="Wo")
            for w_s, w_d in ((Wq_s, Wq_d), (Wk_s, Wk_d), (Wv_s, Wv_d), (Wo_s, Wo_d)):
                nc.sync.dma_start(
                    out=w_s[:].rearrange("p (c n) -> p c n", c=4),
                    in_=w_d.rearrange("(c p) n -> p c n", c=4),
                )
            bq_s = cpool.tile([128, 4], FP32, tag="bq")
            bk_s = cpool.tile([128, 4], FP32, tag="bk")
            bv_s = cpool.tile([128, 4], FP32, tag="bv")
            bo_s = cpool.tile([1, F], F32R, tag="bo")
            for b_s, b_d in ((bq_s, bq_d), (bk_s, bk_d), (bv_s, bv_d)):
                nc.sync.dma_start(
                    out=b_s[:],
                    in_=b_d.rearrange("(c p) -> p c", p=128),
                )
            nc.sync.dma_start(out=bo_s[:], in_=bo_d.rearrange("(o f) -> o f", o=1))
            ones_s = cpool.tile([1, 128], F32R, tag="ones")
            nc.sync.dma_start(out=ones_s[:], in_=ones_d.rearrange("(o f) -> o f", o=1))

            # ---- phase A: load inputs + projections, per batch ----
            qT = []  # [b] -> [128, 4*L] bf16: fout chunk c at cols c*L
            kT = []
            vA = []  # [b] -> [128, 8*(8*65)] bf16: k-tile t at cols t*520, head h at +h*65
            xat = []  # [b] -> [128, 4*L] bf16: hd chunk c at cols c*L
            for b in range(BPC):
                xq_t = xpool.tile([128, 4 * L], F32R, tag="xq")
                xkv_t = xpool.tile([128, 4 * L], F32R, tag="xkv")
                nc.sync.dma_start(
                    out=xq_t[:].rearrange("p (c l) -> p c l", c=4),
                    in_=xqT_d[b].rearrange("(c p) l -> p c l", c=4),
                )
                nc.sync.dma_start(
                    out=xkv_t[:].rearrange("p (c l) -> p c l", c=4),
                    in_=xkvT_d[b].rearrange("(c p) l -> p c l", c=4),
                )
                qT_t = qpool.tile([128, 4 * L], BF16, tag="qT")
                kT_t = qpool.tile([128, 4 * L], BF16, tag="kT")
                vA_t = qpool.tile([128, 8 * 8 * 65], BF16, tag="vA")
                xat_t = qpool.tile([128, 4 * L], BF16, tag="xat")
                qT.append(qT_t)
                kT.append(kT_t)
                vA.append(vA_t)
                xat.append(xat_t)

                # qT / kT: out^T form, W stationary, xT moving
                for dst, w_s, b_s, x_t in (
                    (qT_t, Wq_s, bq_s, xq_t),
                    (kT_t, Wk_s, bk_s, xkv_t),
                ):
                    for fo in range(4):
                        for lc in range(2):
                            pq = psA.tile([128, 512], FP32, tag="ps")
                            for kc in range(4):
                                nc.tensor.matmul(
                                    pq[:],
                                    w_s[:, kc * F + fo * 128 : kc * F + (fo + 1) * 128],
                                    x_t[:, kc * L + lc * 512 : kc * L + (lc + 1) * 512],
                                    start=(kc == 0),
                                    stop=(kc == 3),
                                )
                            nc.scalar.activation(
                                dst[:, fo * L + lc * 512 : fo * L + (lc + 1) * 512],
                                pq[:],
                                Identity,
                                bias=b_s[:, fo : fo + 1],
                            )

                # v natural: xT chunks stationary, Wv moving; assemble v_aug
                for lt in range(8):
                    pv = psA.tile([128, 512], FP32, tag="ps")
                    for kc in range(4):
                        nc.tensor.matmul(
                            pv[:],
                            xkv_t[:, kc * L + lt * 128 : kc * L + (lt + 1) * 128],
                            Wv_s[:, kc * F : (kc + 1) * F],
                            start=(kc == 0),
                            stop=(kc == 3),
                        )
                    base = lt * 520
                    for h in range(8):
                        nc.scalar.copy(
                            vA_t[:, base + h * 65 : base + h * 65 + 64],
                            pv[:, h * 64 : (h + 1) * 64],
                        )
                        nc.gpsimd.memset(vA_t[:, base + h * 65 + 64 : base + h * 65 + 65], 1.0)

            # ---- phase B: attention per head (bias reused across batches) ----
            for h in range(H):
                bias_t = bpool.tile([128, 8 * L], BF16, tag="bias")
                nc.sync.dma_start(
                    out=bias_t[:].rearrange("p (t q) -> p t q", t=8),
                    in_=biasT_d[h].rearrange("(t p) q -> p t q", t=8),
                )
                hp = (h % 2) * 64  # partition offset within fout chunk
                hc = (h // 2) * L  # column offset of fout chunk
                for b in range(BPC):
                    for qc in range(2):
                        q_sl = qT[b][hp : hp + 64, hc + qc * 512 : hc + (qc + 1) * 512]
                        exps = []
                        for kt in range(8):
                            ps = psA.tile([128, 512], FP32, tag="ps")
                            nc.tensor.matmul(
                                ps[:],
                                kT[b][hp : hp + 64, hc + kt * 128 : hc + (kt + 1) * 128],
                                q_sl,
                                start=True,
                                stop=True,
                            )
                            sc = wpool.tile([128, 512], FP32, tag="sc")
                            nc.vector.tensor_tensor(
                                sc[:],
                                ps[:],
                                bias_t[:, kt * L + qc * 512 : kt * L + (qc + 1) * 512],
                                Add,
                            )
                            ex = epool.tile([128, 512], BF16, tag="ex")
                            nc.scalar.activation(ex[:], sc[:], Exp)
                            exps.append(ex)
                        U = psU.tile([65, 512], FP32, tag="u")
                        for kt in range(8):
                            nc.tensor.matmul(
                                U[:],
                                vA[b][:, kt * 520 + h * 65 : kt * 520 + (h + 1) * 65],
                                exps[kt][:],
                                start=(kt == 0),
                                stop=(kt == 7),
                            )
                        recip = wpool.tile([1, 512], F32R, tag="recip")
                        with nc.allow_low_precision(reason="f32r is fp32-width"):
                            nc.vector.reciprocal(recip[:], U[64:65, :])
                        rb = psR.tile([64, 512], FP32, tag="rb")
                        nc.tensor.matmul(
                            rb[:],
                            ones_s[:, :64],
                            recip[:],
                            start=True,
                            stop=True,
                        )
                        rbs = wpool.tile([64, 512], FP32, tag="rbs")
                        nc.scalar.copy(rbs[:], rb[:])
                        x_sl = xat[b][hp : hp + 64, hc + qc * 512 : hc + (qc + 1) * 512]
                        nc.vector.tensor_tensor(x_sl, U[:64, :], rbs[:], Mult)
                        nc.scalar.activation(
                            x_sl, x_sl, Identity, bias=bv_s[hp : hp + 64, h // 2 : h // 2 + 1]
                        )

            # ---- phase C: output projection ----
            for b in range(BPC):
                for lt in range(8):
                    po = psA.tile([128, 512], FP32, tag="ps")
                    for c in range(4):
                        nc.tensor.matmul(
                            po[:],
                            xat[b][:, c * L + lt * 128 : c * L + (lt + 1) * 128],
                            Wo_s[:, c * F : (c + 1) * F],
                            start=(c == 0),
                            stop=False,
                        )
                    nc.tensor.matmul(
                        po[:],
                        ones_s[:, :128],
                        bo_s[:],
                        start=False,
                        stop=True,
                    )
                    os_t = wpool.tile([128, 512], FP32, tag="os")
                    nc.vector.tensor_copy(os_t[:], po[:])
                    nc.sync.dma_start(out=out_d[b, lt * 128 : (lt + 1) * 128, :], in_=os_t[:])

    nc.compile()
    return nc


_NC = None


def _get_nc():
    global _NC
    if _NC is None:
        _NC = _build()
    return _NC


def _prep_in_maps(inputs):
    xq = np.asarray(inputs["inputs_q"], dtype=np.float32)
    xkv = np.asarray(inputs["inputs_kv"], dtype=np.float32)
    Wq = np.asarray(inputs["Wq"], dtype=np.float32) * 0.125
    bq = np.asarray(inputs["bq"], dtype=np.float32) * 0.125
    Wk = np.asarray(inputs["Wk"], dtype=np.float32)
    bk = np.asarray(inputs["bk"], dtype=np.float32)
    Wv = np.asarray(inputs["Wv"], dtype=np.float32)
    bv = np.asarray(inputs["bv"], dtype=np.float32)
    Wo = np.asarray(inputs["Wo"], dtype=np.float32).astype(ml_dtypes.bfloat16)
    bo = np.asarray(inputs["bo"], dtype=np.float32)
    toe = np.asarray(inputs["toeplitz"], dtype=np.float32)

    xqT = np.ascontiguousarray(xq.transpose(0, 2, 1))  # [B, F, L]
    xkvT = np.ascontiguousarray(xkv.transpose(0, 2, 1))

    coords = np.arange(L)
    xi, yi = coords // NY, coords % NY
    dx = xi[:, None] - xi[None, :] + NX
    dy = yi[:, None] - yi[None, :] + NY
    idx = dx * (2 * NY) + dy  # [L(q), L(k)]
    bias = toe[:, idx]  # [H, L(q), L(k)]
    biasT = np.ascontiguousarray(bias.transpose(0, 2, 1)).astype(ml_dtypes.bfloat16)

    in_maps = []
    for i in range(NCORES):
        sl = slice(i * BPC, (i + 1) * BPC)
        in_maps.append(
            {
                "xqT": np.ascontiguousarray(xqT[sl]),
                "xkvT": np.ascontiguousarray(xkvT[sl]),
                "Wq": Wq, "Wk": Wk, "Wv": Wv, "Wo": Wo,
                "bq": bq, "bk": bk, "bv": bv, "bo": bo,
                "biasT": biasT,
                "ones": np.ones(128, dtype=np.float32),
            }
        )
    return in_maps


def _run(inputs, trace=False):
    from concourse.bass_interp import get_hw_module

    nc = _get_nc()
    in_maps = _prep_in_maps(inputs)
    old_m = nc.m
    nc.m = get_hw_module(nc.m)
    try:
        res = run_bass_kernel_spmd(
            nc, in_maps, core_ids=list(range(NCORES)), trace=trace
        )
    finally:
        nc.m = old_m
    out = np.concatenate([r["out"] for r in res.results], axis=0)  # [B, L, F]
    return out.reshape(B, L, H, D), res


def kernel(**inputs) -> np.ndarray:
    out, _ = _run(inputs, trace=False)
    return out
